# revision 21
# baseline (speedup 1.0000x reference)
"""DHPM (deep hidden physics model) forward + derivatives kernel for 8 TRN2 cores.

Math (per point p in R^3, per net n in {u, v}):
  z0 = W0 p + b0 ; h0 = sin(z0) ; c0 = cos(z0)
  z1 = W1 h0 + b1 ; h1 = sin(z1) ; c1 = cos(z1)
  u  = w2.h1 + b2
  directional derivs (d in {x,y,t}):  t0d = W0[:,d]
    z1d = W1 (c0*t0d) = (W1 diag(t0d)) c0       -> h1d = c1*z1d ; u_d = w2.h1d
  second derivs (d in {x,y}):
    z1q = W1 (-h0*t0d^2) = (W1 diag(-t0d^2)) h0
    u_dd = w2.(c1*z1q) - w2.(h1*z1d^2)
  feat = [u, v, u_t, v_t, u_x, u_xx, u_y, u_yy, v_x, v_xx, v_y, v_yy]
  f = MLP_f(feat)  (2 outputs)

Layout: feature-major [feature partitions, batch free].  All matmuls fp32r.
sin/cos via range reduction: weights of "forward" layers pre-scaled by 1/2pi so
PSUM holds s~ = z/2pi; then
  m' = (s~ + b'') + MAGIC            (DVE tensor_scalar, rounds to rnd(s)+MAGIC)
  f' = (m' - MAGIC) - s~             (DVE scalar_tensor_tensor) == rnd(s) - s~
  h  = Sin(f' * -2pi + b)            (ACT)  == sin(z + b - 2pi k), |arg|<=3.93
  c  = Sin(f' * -2pi + b + pi/2)     (ACT)
where b'' = (b + pi/4)/2pi, s = s~ + b''.
"""
import sys
sys.path.insert(0, "/opt/trn_rl_repo")

import numpy as np
import concourse.bacc as bacc
import concourse.mybir as mybir
import concourse.tile as tile
from concourse.bass_utils import run_bass_kernel_spmd

F32 = mybir.dt.float32
F32R = mybir.dt.float32r
AF = mybir.ActivationFunctionType
ALU = mybir.AluOpType

N = 262144
NCORES = 8
NC = N // NCORES          # 32768 points per core
B = 512                   # batch tile (matmul free dim)
NTILES = NC // B          # 64
HID = 200
P0, P1 = 128, 72          # feature partition split of 200
TWO_PI = float(2.0 * np.pi)
QPI = float(np.pi / 4)
HPI = float(np.pi / 2)
MAGIC = float(1.5 * 2 ** 23)
BUFS_Z0, BUFS_Z1, BUFS_FEAT, BUFS_FOUT = 3, 3, 1, 1
BUFS_HC, BUFS_RD = 2, 2
GPS_M1 = True
MP_ACT = True
DEFER_CONTR = False
ORDER = ("a1", "bu", "c1", "a2", "c2", "bv", "c3")

# feat row assignment (matches reference output order)
ROW = {("u", "val"): 0, ("v", "val"): 1,
       ("u", "t"): 2, ("v", "t"): 3,
       ("u", "x"): 4, ("u", "xx"): 5, ("u", "y"): 6, ("u", "yy"): 7,
       ("v", "x"): 8, ("v", "xx"): 9, ("v", "y"): 10, ("v", "yy"): 11}

_CACHE = {}


def _consts(inputs):
    """Fold all constants on the host (float64 -> float32)."""
    c = {}
    for net in ("u", "v", "f"):
        W0 = inputs[f"W{net}0"].astype(np.float64)
        b0 = inputs[f"b{net}0"].astype(np.float64)
        W1 = inputs[f"W{net}1"].astype(np.float64)
        b1 = inputs[f"b{net}1"].astype(np.float64)
        W2 = inputs[f"W{net}2"].astype(np.float64)
        b2 = inputs[f"b{net}2"].astype(np.float64)
        # forward weights pre-scaled by 1/2pi, transposed to lhsT [K, M]
        l0 = (W0 / TWO_PI).T  # [in, 200]
        if net != "f":
            # append b0'' bias row; rhs gets a ones row (host-prepared xT)
            b0r = ((b0 + QPI) / TWO_PI).reshape(1, HID)
            l0 = np.concatenate([b0r, l0], axis=0)  # [4, 200]: ones row first
            c[f"{net}_b1row"] = (((b1 + QPI) / TWO_PI)
                                 .reshape(1, HID).astype(np.float32).copy())
        c[f"{net}_l0T"] = l0.astype(np.float32).copy()
        c[f"{net}_l1T"] = (W1 / TWO_PI).T.astype(np.float32).copy()     # [200, 200]
        for layer, b in (("0", b0), ("1", b1)):
            c[f"{net}_b{layer}r"] = (((b + QPI) / TWO_PI)
                                     .astype(np.float32).reshape(HID, 1).copy())
            c[f"{net}_b{layer}h"] = b.astype(np.float32).reshape(HID, 1).copy()
            c[f"{net}_b{layer}c"] = (b + HPI).astype(np.float32).reshape(HID, 1).copy()
        if net == "f":
            c["f_l2T"] = W2.T.astype(np.float32).copy()                 # [200, 2]
            c["f_b2"] = b2.astype(np.float32).reshape(2, 1).copy()
        else:
            # derivative-pass weights (unscaled)
            W1T = W1.T  # [k, m]
            for d, di in (("x", 0), ("y", 1), ("t", 2)):
                t0 = W0[:, di]  # [200]
                c[f"{net}_B{d}T"] = (t0[:, None] * W1T).astype(np.float32).copy()
            for d, di in (("x", 0), ("y", 1)):
                t0 = W0[:, di]
                c[f"{net}_A{d}T"] = ((-t0 * t0)[:, None] * W1T).astype(np.float32).copy()
            # contraction lhsT [K, 12] per variant per K-split, w2 in one column
            w2 = W2.reshape(HID)
            for var, col, sgn in (("h1", ROW[(net, "val")], 1.0),
                                  ("dx", ROW[(net, "x")], 1.0),
                                  ("dy", ROW[(net, "y")], 1.0),
                                  ("dt", ROW[(net, "t")], 1.0),
                                  ("qx", ROW[(net, "xx")], 1.0),
                                  ("sx", ROW[(net, "xx")], -1.0),
                                  ("qy", ROW[(net, "yy")], 1.0),
                                  ("sy", ROW[(net, "yy")], -1.0)):
                m = np.zeros((HID, 12), dtype=np.float64)
                m[:, col] = sgn * w2
                c[f"{net}_C{var}T"] = m.astype(np.float32).copy()
    # feat bias: rows 0,1 = b2u, b2v
    fb = np.zeros((12, 1), dtype=np.float64)
    fb[0, 0] = inputs["bu2"].astype(np.float64)[0]
    fb[1, 0] = inputs["bv2"].astype(np.float64)[0]
    c["feat_bias"] = fb.astype(np.float32).copy()
    return c


def _build(ntiles, repeat=1):
    """Build the Bacc program for one core processing ntiles*B points."""
    nc = bacc.Bacc("TRN2", target_bir_lowering=False, debug=False)
    npts = ntiles * B

    xT = nc.dram_tensor("xT", [4, npts], F32, kind="ExternalInput")
    out = nc.dram_tensor("out", [14, npts], F32, kind="ExternalOutput")

    # constant dram params
    cd = {}

    def cparam(name, shape):
        cd[name] = nc.dram_tensor(name, list(shape), F32, kind="ExternalInput")
        return cd[name]

    for net in ("u", "v"):
        cparam(f"{net}_l0T", (4, HID))
        cparam(f"{net}_l1T", (HID, HID))
        for d in ("x", "y", "t"):
            cparam(f"{net}_B{d}T", (HID, HID))
        for d in ("x", "y"):
            cparam(f"{net}_A{d}T", (HID, HID))
        for var in ("h1", "dx", "dy", "dt", "qx", "sx", "qy", "sy"):
            cparam(f"{net}_C{var}T", (HID, 12))
        cparam(f"{net}_b1row", (1, HID))
        for layer in ("0", "1"):
            for sfx in ("r", "h", "c"):
                cparam(f"{net}_b{layer}{sfx}", (HID, 1))
    cparam("f_l0T", (12, HID))
    cparam("f_l1T", (HID, HID))
    cparam("f_l2T", (HID, 2))
    for layer in ("0", "1"):
        for sfx in ("r", "h"):
            cparam(f"f_b{layer}{sfx}", (HID, 1))
    cparam("f_b2", (2, 1))
    cparam("feat_bias", (12, 1))

    def r(ap):
        return ap.bitcast(F32R)

    with tile.TileContext(nc) as tc:
        with (
            tc.tile_pool(name="const", bufs=1) as const,
            tc.tile_pool(name="xp", bufs=2) as xp,
            tc.tile_pool(name="hc", bufs=BUFS_HC) as hc,
            tc.tile_pool(name="rd", bufs=BUFS_RD) as rd,
            tc.tile_pool(name="outp", bufs=3) as outp,
            tc.tile_pool(name="pz", bufs=2, space="PSUM") as pz,
            tc.tile_pool(name="pf", bufs=2, space="PSUM") as pf,
        ):
            # ---- load constants to SBUF ----
            C = {}

            def load(name, parts=True, dtype=F32R):
                """Load a [K, M] lhsT const; split partition dim at 128 if needed."""
                t = cd[name]
                k = t.shape[0]
                if k <= P0 or not parts:
                    tt = const.tile([k, t.shape[1]], dtype, name=f"c_{name}")
                    nc.sync.dma_start(out=tt, in_=t[:, :].bitcast(dtype))
                    C[name] = (tt,)
                else:
                    ta = const.tile([P0, t.shape[1]], dtype, name=f"c_{name}a")
                    tb = const.tile([k - P0, t.shape[1]], dtype, name=f"c_{name}b")
                    nc.sync.dma_start(out=ta, in_=t[0:P0, :].bitcast(dtype))
                    nc.sync.dma_start(out=tb, in_=t[P0:k, :].bitcast(dtype))
                    C[name] = (ta, tb)

            def loadb(name):
                """Bias [HID,1] -> two per-partition tiles (fp32)."""
                t = cd[name]
                ta = const.tile([P0, 1], F32, name=f"c_{name}a")
                tb = const.tile([P1, 1], F32, name=f"c_{name}b")
                nc.sync.dma_start(out=ta, in_=t[0:P0, :])
                nc.sync.dma_start(out=tb, in_=t[P0:HID, :])
                C[name] = (ta, tb)

            for net in ("u", "v"):
                load(f"{net}_l0T")
                load(f"{net}_l1T")
                for d in ("x", "y", "t"):
                    load(f"{net}_B{d}T")
                for d in ("x", "y"):
                    load(f"{net}_A{d}T")
                for var in ("h1", "dx", "dy", "dt", "qx", "sx", "qy", "sy"):
                    load(f"{net}_C{var}T")
                for layer in ("0", "1"):
                    for sfx in ("r", "h", "c"):
                        loadb(f"{net}_b{layer}{sfx}")
            load("f_l0T")
            load("f_l1T")
            load("f_l2T")
            for layer in ("0", "1"):
                for sfx in ("r", "h"):
                    loadb(f"f_b{layer}{sfx}")
            fb2 = const.tile([2, 1], F32, name="c_f_b2")
            nc.sync.dma_start(out=fb2, in_=cd["f_b2"][:, :])
            fbias = const.tile([12, 1], F32, name="c_feat_bias")
            nc.sync.dma_start(out=fbias, in_=cd["feat_bias"][:, :])
            for net in ("u", "v"):
                load(f"{net}_b1row")
            magicT = const.tile([P0, 1], F32, name="c_magic")
            nc.vector.memset(magicT, MAGIC)
            negqpi = const.tile([P0, 1], F32, name="c_negqpi")
            nc.vector.memset(negqpi, -QPI)
            posqpi = const.tile([P0, 1], F32, name="c_posqpi")
            nc.vector.memset(posqpi, QPI)

            SPLITS = ((0, P0), (P0, HID))  # (start, end) feature ranges

            def matmul_200(out_tiles, lhsT_pair, rhs_tiles, start, bias_row=None, bias_ones=None):
                """out[200,B] (2 tiles) += lhsT.T @ rhs, K split over rhs tiles."""
                for mi, (ms, me) in enumerate(SPLITS):
                    for ki in range(len(rhs_tiles)):
                        last = ki == len(rhs_tiles) - 1 and bias_row is None
                        nc.tensor.matmul(
                            out_tiles[mi], lhsT_pair[ki][:, ms:me], rhs_tiles[ki],
                            start=start and ki == 0, stop=last)
                    if bias_row is not None:
                        nc.tensor.matmul(out_tiles[mi], bias_row[0][:, ms:me],
                                         bias_ones, start=False, stop=True)

            def sincos(z_tiles, net, layer, want_cos, tag):
                """PSUM tiles -> (h tiles[, c tiles]) via range-reduced Sin.

                u/v nets: PSUM holds s = (z+b+pi/4)/2pi (bias in matmul);
                  m' = ACT Identity(s + MAGIC); f' = DVE (m'-MAGIC)-s;
                  h = Sin(f'*-2pi - pi/4), c = Sin(f'*-2pi + pi/4).
                f net: PSUM holds s~ = z/2pi (no bias);
                  m' = DVE (s~ + b'') + MAGIC; f' = (m'-MAGIC)-s~;
                  h = Sin(f'*-2pi + b).
                """
                pre = "f" if net == "f" else "l"
                hbufs = 2 if net == "f" else 4
                hs, cs = [], []
                for i, zt in enumerate(z_tiles):
                    p = zt.shape[0]
                    mp = rd.tile([p, B], F32, name=f"mp_{tag}{i}", tag=f"mp{i}",
                                 bufs=2)
                    if net == "f":
                        nc.vector.tensor_scalar(mp, zt, C[f"{net}_b{layer}r"][i],
                                                MAGIC, op0=ALU.add, op1=ALU.add)
                    elif MP_ACT:
                        nc.scalar.activation(mp, zt, AF.Identity,
                                             bias=magicT[0:p, :])
                    else:
                        nc.vector.tensor_scalar(mp, zt, MAGIC, None, op0=ALU.add)
                    fp = rd.tile([p, B], F32, name=f"fp_{tag}{i}", tag=f"fp{i}",
                                 bufs=2)
                    nc.vector.scalar_tensor_tensor(
                        fp, mp, MAGIC, zt, op0=ALU.subtract, op1=ALU.subtract)
                    bh = (C[f"{net}_b{layer}h"][i] if net == "f"
                          else negqpi[0:p, :])
                    ht = hc.tile([p, B], F32R, name=f"h_{tag}{i}",
                                 tag=f"h_{pre}{layer}{i}", bufs=hbufs)
                    nc.scalar.activation(ht, fp, AF.Sin, bias=bh, scale=-TWO_PI)
                    hs.append(ht)
                    if want_cos:
                        ct = hc.tile([p, B], F32R, name=f"c_{tag}{i}",
                                     tag=f"c_{pre}{layer}{i}", bufs=hbufs)
                        nc.scalar.activation(ct, fp, AF.Sin, bias=posqpi[0:p, :],
                                             scale=-TWO_PI)
                        cs.append(ct)
                return (hs, cs) if want_cos else hs

            def zpair(tag):
                za = pz.tile([P0, B], F32, name=f"z_{tag}a", tag="z0", bufs=BUFS_Z0)
                zb = pz.tile([P1, B], F32, name=f"z_{tag}b", tag="z1", bufs=BUFS_Z1)
                return [za, zb]

            def contract(net, var, rhs_tiles, feat, start):
                lt = C[f"{net}_C{var}T"]
                for ki in range(2):
                    nc.tensor.matmul(feat, lt[ki], rhs_tiles[ki],
                                     start=start and ki == 0, stop=ki == 1)

            # ---- software-pipelined stages ----
            # SA1(it): x DMA, z0 MMs, L0 sincos   (light PE, heavy ACT/DVE)
            # SA2(it): z1fwd MMs, L1 sincos
            # SB1(it): u-net deriv MMs + TTs + contractions (heavy PE)
            # SB2(it): v-net same + featS copy + DMA
            # SC1/2/3(it): f-net layer chains
            # emission: SA1(it) SC1(it-2) SB1(it-1) SA2(it) SC2(it-2)
            #           SB2(it-1) SC3(it-2)
            st = {}

            def sa1(it):
                s = st[it] = {}
                sl = slice(it * B, (it + 1) * B)
                s["sl"] = sl
                xt = xp.tile([4, B], F32R, name="xt")
                nc.sync.dma_start(out=xt, in_=r(xT[:, sl]))
                s["xt"] = xt
                zz = {}
                for net in ("u", "v"):
                    zz[net] = zpair(f"{net}0_{it}")
                    matmul_200(zz[net], C[f"{net}_l0T"], [xt], start=True)
                for net in ("u", "v"):
                    s[f"{net}_h0"], s[f"{net}_c0"] = sincos(
                        zz[net], net, "0", True, f"{net}0")

            def sa2(it):
                s = st[it]
                zz = {}
                for net in ("u", "v"):
                    zz[net] = zpair(f"{net}1_{it}")
                    matmul_200(zz[net], C[f"{net}_l1T"], s[f"{net}_h0"], start=True,
                               bias_row=C[f"{net}_b1row"],
                               bias_ones=s["xt"][0:1, :])
                for net in ("u", "v"):
                    s[f"{net}_h1"], s[f"{net}_c1"] = sincos(
                        zz[net], net, "1", True, f"{net}1")

            def sb(it, net, first):
                s = st[it]
                h0, c0 = s[f"{net}_h0"], s[f"{net}_c0"]
                h1, c1 = s[f"{net}_h1"], s[f"{net}_c1"]
                if first:
                    s["feat"] = pf.tile([12, B], F32, name="feat",
                                        bufs=BUFS_FEAT)
                feat = s["feat"]
                contract(net, "h1", h1, feat, first)
                # phase 1: all first-order MM groups
                zd = {}
                for d in ("x", "y", "t"):
                    zd[d] = zpair(f"{net}d{d}_{it}")
                    matmul_200(zd[d], C[f"{net}_B{d}T"], c0, start=True)
                # phase 2: consume them (TT + Square), contraction per dir
                sq = {}
                h1ds = {}
                for d in ("x", "y", "t"):
                    h1d = []
                    for i in range(2):
                        p = zd[d][i].shape[0]
                        t = hc.tile([p, B], F32R, name=f"h1d_{net}{d}{i}",
                                    tag=f"h1d{i}" if not DEFER_CONTR else f"h1d{d}{i}",
                                    bufs=3)
                        nc.vector.tensor_mul(t, c1[i], zd[d][i])
                        h1d.append(t)
                        if d != "t":
                            sqt = hc.tile([p, B], F32, name=f"sq_{net}{d}{i}",
                                          tag=f"sq{d}{i}", bufs=3)
                            nc.scalar.square(sqt, zd[d][i])
                            sq.setdefault(d, []).append(sqt)
                    h1ds[d] = h1d
                    if not DEFER_CONTR:
                        contract(net, f"d{d}", h1d, feat, False)
                if DEFER_CONTR:
                    for d in ("x", "y", "t"):
                        contract(net, f"d{d}", h1ds[d], feat, False)
                # phase 3: second-order MM groups
                zq = {}
                for d in ("x", "y"):
                    zq[d] = zpair(f"{net}q{d}_{it}")
                    matmul_200(zq[d], C[f"{net}_A{d}T"], h0, start=True)
                # phase 4: consume + contract
                for d in ("x", "y"):
                    m2 = []
                    m1 = []
                    for i in range(2):
                        p = zq[d][i].shape[0]
                        t2 = hc.tile([p, B], F32R, name=f"m2_{net}{d}{i}",
                                     tag=f"m2{i}", bufs=3)
                        nc.vector.tensor_mul(t2, c1[i], zq[d][i])
                        m2.append(t2)
                        t1 = hc.tile([p, B], F32R, name=f"m1_{net}{d}{i}",
                                     tag=f"m1{i}", bufs=3)
                        if GPS_M1:
                            nc.gpsimd.tensor_mul(t1, sq[d][i], h1[i])
                        else:
                            nc.vector.tensor_mul(t1, sq[d][i], h1[i])
                        m1.append(t1)
                    contract(net, f"q{d}", m2, feat, False)
                    contract(net, f"s{d}", m1, feat, False)

            def sb2_tail(it):
                s = st[it]
                featS = outp.tile([12, B], F32R, name="featS", bufs=3)
                nc.scalar.activation(featS, s["feat"], AF.Identity, bias=fbias)
                nc.sync.dma_start(out=r(out[0:12, s["sl"]]), in_=featS)
                s["featS"] = featS

            def sc1(it):
                s = st[it]
                zf0 = zpair(f"f0_{it}")
                matmul_200(zf0, C["f_l0T"], [s["featS"]], start=True)
                s["hf0"] = sincos(zf0, "f", "0", False, "f0")

            def sc2(it):
                s = st[it]
                zf1 = zpair(f"f1_{it}")
                matmul_200(zf1, C["f_l1T"], s["hf0"], start=True)
                s["hf1"] = sincos(zf1, "f", "1", False, "f1")

            def sc3(it):
                s = st[it]
                fout = pf.tile([2, B], F32, name="fout", tag="fout",
                               bufs=BUFS_FOUT)
                for ki in range(2):
                    nc.tensor.matmul(fout, C["f_l2T"][ki], s["hf1"][ki],
                                     start=ki == 0, stop=ki == 1)
                foutS = outp.tile([2, B], F32, name="foutS")
                nc.scalar.activation(foutS, fout, AF.Identity, bias=fb2)
                nc.sync.dma_start(out=out[12:14, s["sl"]], in_=foutS)
                del st[it]

            for rep in range(repeat):
                import os
                order = ORDER
                for it in range(ntiles + 2):
                    for step in order:
                        if step == "a1" and it < ntiles:
                            sa1(it)
                        elif step == "a2" and it < ntiles:
                            sa2(it)
                        elif step == "bu" and 0 <= it - 1 < ntiles:
                            sb(it - 1, "u", True)
                        elif step == "bv" and 0 <= it - 1 < ntiles:
                            sb(it - 1, "v", False)
                            sb2_tail(it - 1)
                        elif step == "c1" and 0 <= it - 2 < ntiles:
                            sc1(it - 2)
                        elif step == "c2" and 0 <= it - 2 < ntiles:
                            sc2(it - 2)
                        elif step == "c3" and 0 <= it - 2 < ntiles:
                            sc3(it - 2)

    nc.compile()
    return nc


def kernel(**inputs):
    key = "full"
    if key not in _CACHE:
        _CACHE[key] = _build(NTILES)
    nc = _CACHE[key]

    c = _consts(inputs)
    x = np.asarray(inputs["inputs"], dtype=np.float32)  # [N, 3]

    in_maps = []
    for core in range(NCORES):
        shard = x[core * NC:(core + 1) * NC]            # [NC, 3]
        xt = np.empty((4, NC), dtype=np.float32)
        xt[0] = 1.0
        xt[1:4] = shard.T
        m = {"xT": xt}
        m.update(c)
        in_maps.append(m)

    res = run_bass_kernel_spmd(nc, in_maps, core_ids=list(range(NCORES)))
    outs = [res.results[i]["out"] for i in range(NCORES)]  # each [14, NC]
    full = np.concatenate(outs, axis=1)                    # [14, N]
    return tuple(np.ascontiguousarray(full[j]).reshape(N, 1)
                 for j in range(14))


if __name__ == "__main__":
    pass


# revision 22
# speedup vs baseline: 1.0621x; 1.0621x over previous
"""DHPM (deep hidden physics model) forward + derivatives kernel for 8 TRN2 cores.

Math (per point p in R^3, per net n in {u, v}):
  z0 = W0 p + b0 ; h0 = sin(z0) ; c0 = cos(z0)
  z1 = W1 h0 + b1 ; h1 = sin(z1) ; c1 = cos(z1)
  u  = w2.h1 + b2
  directional derivs (d in {x,y,t}):  t0d = W0[:,d]
    z1d = W1 (c0*t0d) = (W1 diag(t0d)) c0       -> h1d = c1*z1d ; u_d = w2.h1d
  second derivs (d in {x,y}):
    z1q = W1 (-h0*t0d^2) = (W1 diag(-t0d^2)) h0
    u_dd = w2.(c1*z1q) - w2.(h1*z1d^2)
  feat = [u, v, u_t, v_t, u_x, u_xx, u_y, u_yy, v_x, v_xx, v_y, v_yy]
  f = MLP_f(feat)  (2 outputs)

Layout: feature-major [feature partitions, batch free].  All matmuls fp32r.
sin/cos via range reduction: weights of "forward" layers pre-scaled by 1/2pi so
PSUM holds s~ = z/2pi; then
  m' = (s~ + b'') + MAGIC            (DVE tensor_scalar, rounds to rnd(s)+MAGIC)
  f' = (m' - MAGIC) - s~             (DVE scalar_tensor_tensor) == rnd(s) - s~
  h  = Sin(f' * -2pi + b)            (ACT)  == sin(z + b - 2pi k), |arg|<=3.93
  c  = Sin(f' * -2pi + b + pi/2)     (ACT)
where b'' = (b + pi/4)/2pi, s = s~ + b''.
"""
import sys
sys.path.insert(0, "/opt/trn_rl_repo")

import numpy as np
import concourse.bacc as bacc
import concourse.mybir as mybir
import concourse.tile as tile
from concourse.bass_utils import run_bass_kernel_spmd

F32 = mybir.dt.float32
F32R = mybir.dt.float32r
AF = mybir.ActivationFunctionType
ALU = mybir.AluOpType

N = 262144
NCORES = 8
NC = N // NCORES          # 32768 points per core
B = 512                   # batch tile (matmul free dim)
NTILES = NC // B          # 64
HID = 200
P0, P1 = 128, 72          # feature partition split of 200
TWO_PI = float(2.0 * np.pi)
QPI = float(np.pi / 4)
HPI = float(np.pi / 2)
MAGIC = float(1.5 * 2 ** 23)
BUFS_Z0, BUFS_Z1, BUFS_FEAT, BUFS_FOUT = 4, 2, 1, 1
BUFS_HC, BUFS_RD = 2, 2
GPS_M1 = True
MP_ACT = True
DEFER_CONTR = False
ORDER = ("a1", "bu", "c1", "a2", "c2", "bv", "c3")

# feat row assignment (matches reference output order)
ROW = {("u", "val"): 0, ("v", "val"): 1,
       ("u", "t"): 2, ("v", "t"): 3,
       ("u", "x"): 4, ("u", "xx"): 5, ("u", "y"): 6, ("u", "yy"): 7,
       ("v", "x"): 8, ("v", "xx"): 9, ("v", "y"): 10, ("v", "yy"): 11}

_CACHE = {}


def _consts(inputs):
    """Fold all constants on the host (float64 -> float32)."""
    c = {}
    for net in ("u", "v", "f"):
        W0 = inputs[f"W{net}0"].astype(np.float64)
        b0 = inputs[f"b{net}0"].astype(np.float64)
        W1 = inputs[f"W{net}1"].astype(np.float64)
        b1 = inputs[f"b{net}1"].astype(np.float64)
        W2 = inputs[f"W{net}2"].astype(np.float64)
        b2 = inputs[f"b{net}2"].astype(np.float64)
        # forward weights pre-scaled by 1/2pi, transposed to lhsT [K, M]
        l0 = (W0 / TWO_PI).T  # [in, 200]
        if net != "f":
            # append b0'' bias row; rhs gets a ones row (host-prepared xT)
            b0r = ((b0 + QPI) / TWO_PI).reshape(1, HID)
            l0 = np.concatenate([b0r, l0], axis=0)  # [4, 200]: ones row first
            c[f"{net}_b1row"] = (((b1 + QPI) / TWO_PI)
                                 .reshape(1, HID).astype(np.float32).copy())
        c[f"{net}_l0T"] = l0.astype(np.float32).copy()
        c[f"{net}_l1T"] = (W1 / TWO_PI).T.astype(np.float32).copy()     # [200, 200]
        for layer, b in (("0", b0), ("1", b1)):
            c[f"{net}_b{layer}r"] = (((b + QPI) / TWO_PI)
                                     .astype(np.float32).reshape(HID, 1).copy())
            c[f"{net}_b{layer}h"] = b.astype(np.float32).reshape(HID, 1).copy()
            c[f"{net}_b{layer}c"] = (b + HPI).astype(np.float32).reshape(HID, 1).copy()
        if net == "f":
            c["f_l2T"] = W2.T.astype(np.float32).copy()                 # [200, 2]
            c["f_b2"] = b2.astype(np.float32).reshape(2, 1).copy()
        else:
            # derivative-pass weights (unscaled)
            W1T = W1.T  # [k, m]
            for d, di in (("x", 0), ("y", 1), ("t", 2)):
                t0 = W0[:, di]  # [200]
                c[f"{net}_B{d}T"] = (t0[:, None] * W1T).astype(np.float32).copy()
            for d, di in (("x", 0), ("y", 1)):
                t0 = W0[:, di]
                c[f"{net}_A{d}T"] = ((-t0 * t0)[:, None] * W1T).astype(np.float32).copy()
            # contraction lhsT [K, 12] per variant per K-split, w2 in one column
            w2 = W2.reshape(HID)
            for var, col, sgn in (("h1", ROW[(net, "val")], 1.0),
                                  ("dx", ROW[(net, "x")], 1.0),
                                  ("dy", ROW[(net, "y")], 1.0),
                                  ("dt", ROW[(net, "t")], 1.0),
                                  ("qx", ROW[(net, "xx")], 1.0),
                                  ("sx", ROW[(net, "xx")], -1.0),
                                  ("qy", ROW[(net, "yy")], 1.0),
                                  ("sy", ROW[(net, "yy")], -1.0)):
                m = np.zeros((HID, 12), dtype=np.float64)
                m[:, col] = sgn * w2
                c[f"{net}_C{var}T"] = m.astype(np.float32).copy()
    # feat bias: rows 0,1 = b2u, b2v
    fb = np.zeros((12, 1), dtype=np.float64)
    fb[0, 0] = inputs["bu2"].astype(np.float64)[0]
    fb[1, 0] = inputs["bv2"].astype(np.float64)[0]
    c["feat_bias"] = fb.astype(np.float32).copy()
    return c


def _build(ntiles, repeat=1):
    """Build the Bacc program for one core processing ntiles*B points."""
    nc = bacc.Bacc("TRN2", target_bir_lowering=False, debug=False)
    npts = ntiles * B

    xT = nc.dram_tensor("xT", [4, npts], F32, kind="ExternalInput")
    out = nc.dram_tensor("out", [14, npts], F32, kind="ExternalOutput")

    # constant dram params
    cd = {}

    def cparam(name, shape):
        cd[name] = nc.dram_tensor(name, list(shape), F32, kind="ExternalInput")
        return cd[name]

    for net in ("u", "v"):
        cparam(f"{net}_l0T", (4, HID))
        cparam(f"{net}_l1T", (HID, HID))
        for d in ("x", "y", "t"):
            cparam(f"{net}_B{d}T", (HID, HID))
        for d in ("x", "y"):
            cparam(f"{net}_A{d}T", (HID, HID))
        for var in ("h1", "dx", "dy", "dt", "qx", "sx", "qy", "sy"):
            cparam(f"{net}_C{var}T", (HID, 12))
        cparam(f"{net}_b1row", (1, HID))
        for layer in ("0", "1"):
            for sfx in ("r", "h", "c"):
                cparam(f"{net}_b{layer}{sfx}", (HID, 1))
    cparam("f_l0T", (12, HID))
    cparam("f_l1T", (HID, HID))
    cparam("f_l2T", (HID, 2))
    for layer in ("0", "1"):
        for sfx in ("r", "h"):
            cparam(f"f_b{layer}{sfx}", (HID, 1))
    cparam("f_b2", (2, 1))
    cparam("feat_bias", (12, 1))

    def r(ap):
        return ap.bitcast(F32R)

    with tile.TileContext(nc) as tc:
        with (
            tc.tile_pool(name="const", bufs=1) as const,
            tc.tile_pool(name="xp", bufs=2) as xp,
            tc.tile_pool(name="hc", bufs=BUFS_HC) as hc,
            tc.tile_pool(name="rd", bufs=BUFS_RD) as rd,
            tc.tile_pool(name="outp", bufs=3) as outp,
            tc.tile_pool(name="pz", bufs=2, space="PSUM") as pz,
            tc.tile_pool(name="pf", bufs=2, space="PSUM") as pf,
        ):
            # ---- load constants to SBUF ----
            C = {}

            def load(name, parts=True, dtype=F32R):
                """Load a [K, M] lhsT const; split partition dim at 128 if needed."""
                t = cd[name]
                k = t.shape[0]
                if k <= P0 or not parts:
                    tt = const.tile([k, t.shape[1]], dtype, name=f"c_{name}")
                    nc.sync.dma_start(out=tt, in_=t[:, :].bitcast(dtype))
                    C[name] = (tt,)
                else:
                    ta = const.tile([P0, t.shape[1]], dtype, name=f"c_{name}a")
                    tb = const.tile([k - P0, t.shape[1]], dtype, name=f"c_{name}b")
                    nc.sync.dma_start(out=ta, in_=t[0:P0, :].bitcast(dtype))
                    nc.sync.dma_start(out=tb, in_=t[P0:k, :].bitcast(dtype))
                    C[name] = (ta, tb)

            def loadb(name):
                """Bias [HID,1] -> two per-partition tiles (fp32)."""
                t = cd[name]
                ta = const.tile([P0, 1], F32, name=f"c_{name}a")
                tb = const.tile([P1, 1], F32, name=f"c_{name}b")
                nc.sync.dma_start(out=ta, in_=t[0:P0, :])
                nc.sync.dma_start(out=tb, in_=t[P0:HID, :])
                C[name] = (ta, tb)

            for net in ("u", "v"):
                load(f"{net}_l0T")
                load(f"{net}_l1T")
                for d in ("x", "y", "t"):
                    load(f"{net}_B{d}T")
                for d in ("x", "y"):
                    load(f"{net}_A{d}T")
                for var in ("h1", "dx", "dy", "dt", "qx", "sx", "qy", "sy"):
                    load(f"{net}_C{var}T")
                for layer in ("0", "1"):
                    for sfx in ("r", "h", "c"):
                        loadb(f"{net}_b{layer}{sfx}")
            load("f_l0T")
            load("f_l1T")
            load("f_l2T")
            for layer in ("0", "1"):
                for sfx in ("r", "h"):
                    loadb(f"f_b{layer}{sfx}")
            fb2 = const.tile([2, 1], F32, name="c_f_b2")
            nc.sync.dma_start(out=fb2, in_=cd["f_b2"][:, :])
            fbias = const.tile([12, 1], F32, name="c_feat_bias")
            nc.sync.dma_start(out=fbias, in_=cd["feat_bias"][:, :])
            for net in ("u", "v"):
                load(f"{net}_b1row")
            magicT = const.tile([P0, 1], F32, name="c_magic")
            nc.vector.memset(magicT, MAGIC)
            negqpi = const.tile([P0, 1], F32, name="c_negqpi")
            nc.vector.memset(negqpi, -QPI)
            posqpi = const.tile([P0, 1], F32, name="c_posqpi")
            nc.vector.memset(posqpi, QPI)

            SPLITS = ((0, P0), (P0, HID))  # (start, end) feature ranges

            def matmul_200(out_tiles, lhsT_pair, rhs_tiles, start, bias_row=None, bias_ones=None):
                """out[200,B] (2 tiles) += lhsT.T @ rhs, K split over rhs tiles."""
                for mi, (ms, me) in enumerate(SPLITS):
                    for ki in range(len(rhs_tiles)):
                        last = ki == len(rhs_tiles) - 1 and bias_row is None
                        nc.tensor.matmul(
                            out_tiles[mi], lhsT_pair[ki][:, ms:me], rhs_tiles[ki],
                            start=start and ki == 0, stop=last)
                    if bias_row is not None:
                        nc.tensor.matmul(out_tiles[mi], bias_row[0][:, ms:me],
                                         bias_ones, start=False, stop=True)

            def sincos(z_tiles, net, layer, want_cos, tag):
                """PSUM tiles -> (h tiles[, c tiles]) via range-reduced Sin.

                u/v nets: PSUM holds s = (z+b+pi/4)/2pi (bias in matmul);
                  m' = ACT Identity(s + MAGIC); f' = DVE (m'-MAGIC)-s;
                  h = Sin(f'*-2pi - pi/4), c = Sin(f'*-2pi + pi/4).
                f net: PSUM holds s~ = z/2pi (no bias);
                  m' = DVE (s~ + b'') + MAGIC; f' = (m'-MAGIC)-s~;
                  h = Sin(f'*-2pi + b).
                """
                pre = "f" if net == "f" else "l"
                hbufs = 2 if net == "f" else 4
                hs, cs = [], []
                for i, zt in enumerate(z_tiles):
                    p = zt.shape[0]
                    mp = rd.tile([p, B], F32, name=f"mp_{tag}{i}", tag=f"mp{i}",
                                 bufs=2)
                    if net == "f":
                        nc.vector.tensor_scalar(mp, zt, C[f"{net}_b{layer}r"][i],
                                                MAGIC, op0=ALU.add, op1=ALU.add)
                    elif MP_ACT:
                        nc.scalar.activation(mp, zt, AF.Identity,
                                             bias=magicT[0:p, :])
                    else:
                        nc.vector.tensor_scalar(mp, zt, MAGIC, None, op0=ALU.add)
                    fp = rd.tile([p, B], F32, name=f"fp_{tag}{i}", tag=f"fp{i}",
                                 bufs=2)
                    nc.vector.scalar_tensor_tensor(
                        fp, mp, MAGIC, zt, op0=ALU.subtract, op1=ALU.subtract)
                    bh = (C[f"{net}_b{layer}h"][i] if net == "f"
                          else negqpi[0:p, :])
                    ht = hc.tile([p, B], F32R, name=f"h_{tag}{i}",
                                 tag=f"h_{pre}{layer}{i}", bufs=hbufs)
                    nc.scalar.activation(ht, fp, AF.Sin, bias=bh, scale=-TWO_PI)
                    hs.append(ht)
                    if want_cos:
                        ct = hc.tile([p, B], F32R, name=f"c_{tag}{i}",
                                     tag=f"c_{pre}{layer}{i}", bufs=hbufs)
                        nc.scalar.activation(ct, fp, AF.Sin, bias=posqpi[0:p, :],
                                             scale=-TWO_PI)
                        cs.append(ct)
                return (hs, cs) if want_cos else hs

            def zpair(tag):
                za = pz.tile([P0, B], F32, name=f"z_{tag}a", tag="z0", bufs=BUFS_Z0)
                zb = pz.tile([P1, B], F32, name=f"z_{tag}b", tag="z1", bufs=BUFS_Z1)
                return [za, zb]

            def contract(net, var, rhs_tiles, feat, start):
                lt = C[f"{net}_C{var}T"]
                for ki in range(2):
                    nc.tensor.matmul(feat, lt[ki], rhs_tiles[ki],
                                     start=start and ki == 0, stop=ki == 1)

            # ---- software-pipelined stages ----
            # SA1(it): x DMA, z0 MMs, L0 sincos   (light PE, heavy ACT/DVE)
            # SA2(it): z1fwd MMs, L1 sincos
            # SB1(it): u-net deriv MMs + TTs + contractions (heavy PE)
            # SB2(it): v-net same + featS copy + DMA
            # SC1/2/3(it): f-net layer chains
            # emission: SA1(it) SC1(it-2) SB1(it-1) SA2(it) SC2(it-2)
            #           SB2(it-1) SC3(it-2)
            st = {}

            def sa1(it):
                s = st[it] = {}
                sl = slice(it * B, (it + 1) * B)
                s["sl"] = sl
                xt = xp.tile([4, B], F32R, name="xt")
                nc.sync.dma_start(out=xt, in_=r(xT[:, sl]))
                s["xt"] = xt
                zz = {}
                for net in ("u", "v"):
                    zz[net] = zpair(f"{net}0_{it}")
                    matmul_200(zz[net], C[f"{net}_l0T"], [xt], start=True)
                for net in ("u", "v"):
                    s[f"{net}_h0"], s[f"{net}_c0"] = sincos(
                        zz[net], net, "0", True, f"{net}0")

            def sa2(it):
                s = st[it]
                zz = {}
                for net in ("u", "v"):
                    zz[net] = zpair(f"{net}1_{it}")
                    matmul_200(zz[net], C[f"{net}_l1T"], s[f"{net}_h0"], start=True,
                               bias_row=C[f"{net}_b1row"],
                               bias_ones=s["xt"][0:1, :])
                for net in ("u", "v"):
                    s[f"{net}_h1"], s[f"{net}_c1"] = sincos(
                        zz[net], net, "1", True, f"{net}1")

            def sb(it, net, first):
                s = st[it]
                h0, c0 = s[f"{net}_h0"], s[f"{net}_c0"]
                h1, c1 = s[f"{net}_h1"], s[f"{net}_c1"]
                if first:
                    s["feat"] = pf.tile([12, B], F32, name="feat",
                                        bufs=BUFS_FEAT)
                feat = s["feat"]
                contract(net, "h1", h1, feat, first)
                # phase 1: all first-order MM groups
                zd = {}
                for d in ("x", "y", "t"):
                    zd[d] = zpair(f"{net}d{d}_{it}")
                    matmul_200(zd[d], C[f"{net}_B{d}T"], c0, start=True)
                # phase 2: consume them (TT + Square), contraction per dir
                sq = {}
                h1ds = {}
                for d in ("x", "y", "t"):
                    h1d = []
                    for i in range(2):
                        p = zd[d][i].shape[0]
                        t = hc.tile([p, B], F32R, name=f"h1d_{net}{d}{i}",
                                    tag=f"h1d{i}" if not DEFER_CONTR else f"h1d{d}{i}",
                                    bufs=3)
                        nc.vector.tensor_mul(t, c1[i], zd[d][i])
                        h1d.append(t)
                        if d != "t":
                            sqt = hc.tile([p, B], F32, name=f"sq_{net}{d}{i}",
                                          tag=f"sq{d}{i}", bufs=3)
                            nc.scalar.square(sqt, zd[d][i])
                            sq.setdefault(d, []).append(sqt)
                    h1ds[d] = h1d
                    if not DEFER_CONTR:
                        contract(net, f"d{d}", h1d, feat, False)
                if DEFER_CONTR:
                    for d in ("x", "y", "t"):
                        contract(net, f"d{d}", h1ds[d], feat, False)
                # phase 3: second-order MM groups
                zq = {}
                for d in ("x", "y"):
                    zq[d] = zpair(f"{net}q{d}_{it}")
                    matmul_200(zq[d], C[f"{net}_A{d}T"], h0, start=True)
                # phase 4: consume + contract
                for d in ("x", "y"):
                    m2 = []
                    m1 = []
                    for i in range(2):
                        p = zq[d][i].shape[0]
                        t2 = hc.tile([p, B], F32R, name=f"m2_{net}{d}{i}",
                                     tag=f"m2{i}", bufs=3)
                        nc.vector.tensor_mul(t2, c1[i], zq[d][i])
                        m2.append(t2)
                        t1 = hc.tile([p, B], F32R, name=f"m1_{net}{d}{i}",
                                     tag=f"m1{i}", bufs=3)
                        if GPS_M1:
                            nc.gpsimd.tensor_mul(t1, sq[d][i], h1[i])
                        else:
                            nc.vector.tensor_mul(t1, sq[d][i], h1[i])
                        m1.append(t1)
                    contract(net, f"q{d}", m2, feat, False)
                    contract(net, f"s{d}", m1, feat, False)

            def sb2_tail(it):
                s = st[it]
                featS = outp.tile([12, B], F32R, name="featS", bufs=3)
                nc.scalar.activation(featS, s["feat"], AF.Identity, bias=fbias)
                nc.sync.dma_start(out=r(out[0:12, s["sl"]]), in_=featS)
                s["featS"] = featS

            def sc1(it):
                s = st[it]
                zf0 = zpair(f"f0_{it}")
                matmul_200(zf0, C["f_l0T"], [s["featS"]], start=True)
                s["hf0"] = sincos(zf0, "f", "0", False, "f0")

            def sc2(it):
                s = st[it]
                zf1 = zpair(f"f1_{it}")
                matmul_200(zf1, C["f_l1T"], s["hf0"], start=True)
                s["hf1"] = sincos(zf1, "f", "1", False, "f1")

            def sc3(it):
                s = st[it]
                fout = pf.tile([2, B], F32, name="fout", tag="fout",
                               bufs=BUFS_FOUT)
                for ki in range(2):
                    nc.tensor.matmul(fout, C["f_l2T"][ki], s["hf1"][ki],
                                     start=ki == 0, stop=ki == 1)
                foutS = outp.tile([2, B], F32, name="foutS")
                nc.scalar.activation(foutS, fout, AF.Identity, bias=fb2)
                nc.sync.dma_start(out=out[12:14, s["sl"]], in_=foutS)
                del st[it]

            for rep in range(repeat):
                import os
                order = ORDER
                for it in range(ntiles + 2):
                    for step in order:
                        if step == "a1" and it < ntiles:
                            sa1(it)
                        elif step == "a2" and it < ntiles:
                            sa2(it)
                        elif step == "bu" and 0 <= it - 1 < ntiles:
                            sb(it - 1, "u", True)
                        elif step == "bv" and 0 <= it - 1 < ntiles:
                            sb(it - 1, "v", False)
                            sb2_tail(it - 1)
                        elif step == "c1" and 0 <= it - 2 < ntiles:
                            sc1(it - 2)
                        elif step == "c2" and 0 <= it - 2 < ntiles:
                            sc2(it - 2)
                        elif step == "c3" and 0 <= it - 2 < ntiles:
                            sc3(it - 2)

    nc.compile()
    return nc


def kernel(**inputs):
    key = "full"
    if key not in _CACHE:
        _CACHE[key] = _build(NTILES)
    nc = _CACHE[key]

    c = _consts(inputs)
    x = np.asarray(inputs["inputs"], dtype=np.float32)  # [N, 3]

    in_maps = []
    for core in range(NCORES):
        shard = x[core * NC:(core + 1) * NC]            # [NC, 3]
        xt = np.empty((4, NC), dtype=np.float32)
        xt[0] = 1.0
        xt[1:4] = shard.T
        m = {"xT": xt}
        m.update(c)
        in_maps.append(m)

    res = run_bass_kernel_spmd(nc, in_maps, core_ids=list(range(NCORES)))
    outs = [res.results[i]["out"] for i in range(NCORES)]  # each [14, NC]
    full = np.concatenate(outs, axis=1)                    # [14, N]
    return tuple(np.ascontiguousarray(full[j]).reshape(N, 1)
                 for j in range(14))


if __name__ == "__main__":
    pass
